# revision 18
# baseline (speedup 1.0000x reference)
"""CoxTime loss kernel for 8 Trainium2 NeuronCores (v4: fp8 + pair-sum).

Host-side layout transform: each core's 32768-row logits shard is sorted
by label descending, so the risk set for column k is a row PREFIX of the
sorted shard.  Per 4096-row chunk g only columns 0..max_label(chunk) are
kept (truncation ~halves the work).  The widest chunks are assigned to
the Activation engine and shipped as fp8 e4m3 (native exp input, half
the HBM bytes); the rest ship as fp16 for the DVE's Schraudolph exp
(round(1477.32*x + 15301.06) as int16 IS fp16(~e^x), one 4x-mode
tensor_scalar).  A 2x-mode tensor_tensor then folds adjacent 128-row
tiles (pair-sum), halving the PE stream.  The PE reduces each chunk with
an indicator-column stationary into PSUM row g: C[256-row block, column]
partial sums.  Host finish: per column k, prefix-sum full blocks of C
plus an exact exp() correction for the <=255 boundary rows, then event
counts/numerators, the log and the scalar loss (O(B)+O(K) host work).
"""

import numpy as np

import concourse.bacc as bacc
import concourse.bass as bass
import concourse.mybir as mybir
import concourse.tile as tile
from concourse.bass_utils import run_bass_kernel_spmd

B = 262144
K = 128
NCORES = 8
BC = B // NCORES          # rows per core
P = 128                   # partitions
NT = BC // P              # 256 row-tiles per core
TPB = 16                  # row-tiles per chunk
NCH = NT // TPB           # 16 chunks per core
NPAIR = TPB // 2          # tile-pairs per chunk (pair-summed blocks)
NBANK = 2                 # PSUM banks: piece i of a chunk -> bank i
PIECE = 512               # moving-free elems per matmul (= 1 PSUM bank)

f32 = mybir.dt.float32
f16 = mybir.dt.float16
fp8 = mybir.dt.float8e4
i16 = mybir.dt.int16

# Schraudolph fp16 exp: i16 bits of fp16(2^t) ~= 1024*(t + 15 - 0.05757)
SCH_A = 1024.0 * 1.4426950408889634      # 1024*log2(e)
SCH_B = 1024.0 * (15.0 - 0.05757)

LAST_EXEC_NS = None
LAST_TRACE = None
LAST_PROFILE_JSON = None


def _plan(labels):
    """Per-core sort, shared chunk widths, act/DVE split, DMA groups."""
    perms, ls_all = [], []
    for i in range(NCORES):
        lab = labels[i * BC:(i + 1) * BC]
        perm = np.argsort(-lab, kind="stable")
        perms.append(perm)
        ls_all.append(lab[perm])
    ls_all = np.stack(ls_all)                      # (NCORES, BC) descending
    W = []
    for g in range(NCH):
        hi = int(ls_all[:, g * TPB * P:(g + 1) * TPB * P].max())
        W.append(min(K, hi + 1 + ((hi + 1) & 1)))  # round W up to even
    E = sum(TPB * w for w in W)                    # flat elems per partition
    # act (fp8) elem budget: balance measured act rate (1.044 ns/elem on
    # fp8) against the DVE's Schraudolph (0.316) + pair-sum (0.84) load
    A = (0.316 * E + 0.42 * E + 1200.0 - 2640.0) / 1.36
    m, rem = [], A
    for g in range(NCH):                           # widest chunks first
        mg = int(min(NPAIR, max(0.0, rem // (2 * W[g]))))
        m.append(mg)
        rem -= mg * 2 * W[g]
    off8 = np.cumsum([0] + [2 * W[g] * m[g] for g in range(NCH)]).tolist()
    off16 = np.cumsum([0] + [TPB * W[g] - 2 * W[g] * m[g]
                             for g in range(NCH)]).tolist()
    L8, L16 = off8[-1], off16[-1]

    def ranges(offs, nsplit):
        tot = offs[-1]
        cuts, res, s = [tot * (i + 1) / nsplit for i in range(nsplit)], [], 0
        for c in cuts:
            e = min(range(NCH + 1), key=lambda g: abs(offs[g] - c))
            if e > s:
                res.append((s, e))
                s = e
        if s < NCH:
            res.append((s, NCH))
        return res

    g8 = ranges(off8, 4)                           # fp8 DMA chunk ranges
    g16 = ranges(off16, 3)                         # fp16 DMA chunk ranges
    return perms, ls_all, W, m, off8, off16, L8, L16, g8, g16


def build_nc(plan):
    W, m, off8, off16, L8, L16, g8, g16 = plan
    nc = bacc.Bacc("TRN2", target_bir_lowering=False)
    packed8 = nc.declare_dram_parameter("packed8", [P, max(L8, 2)], fp8,
                                        isOutput=False)
    packed16 = nc.declare_dram_parameter("packed16", [P, max(L16, 2)], f16,
                                         isOutput=False)
    CW = NCH * NCH + PIECE
    consts = nc.declare_dram_parameter("consts", [P, CW], f16, isOutput=False)
    out = nc.declare_dram_parameter("out", [NCH, NBANK * PIECE], f16,
                                    isOutput=True)

    Lg = [NPAIR * W[g] for g in range(NCH)]        # pair-summed stream len
    npieces = [(Lg[g] + PIECE - 1) // PIECE for g in range(NCH)]
    last_chunk = [max(g for g in range(NCH) if npieces[g] > b)
                  for b in range(NBANK)]
    # emit each fp16 group's Schraudolph right before its first consumer
    # chunk (first chunk in range with a DVE part) so earlier act-chunk
    # pair-sums aren't head-of-line blocked on the DVE queue
    g16_first = {}
    for s, e in g16:
        first = next((g for g in range(s, e) if m[g] < NPAIR), s)
        g16_first[first] = (s, e)

    with tile.TileContext(nc) as tc:
        with (
            tc.tile_pool(name="const", bufs=1) as cpool,
            tc.tile_pool(name="psum", bufs=1, space="PSUM") as pspool,
        ):
            cst = cpool.tile([P, CW], f16)
            nc.sync.dma_start(out=cst[:], in_=consts.ap())
            zcols = cst[:, NCH * NCH:NCH * NCH + NCH]       # [P,16] zeros
            zmove = cst[:, NCH * NCH:NCH * NCH + PIECE]     # [P,512] zeros
            stage = cpool.tile([NCH, NBANK * PIECE], f16)

            lt8 = cpool.tile([P, max(L8, 2)], fp8)
            lt16 = cpool.tile([P, max(L16, 2)], f16)
            s8 = cpool.tile([P, max(L8, 2)], f16)           # act exp out
            s16 = cpool.tile([P, max(L16, 2)], f16)         # dve exp out
            rhs2all = cpool.tile([P, sum(Lg)], f16)
            r2off = np.cumsum([0] + Lg).tolist()
            rhs2 = [rhs2all[:, r2off[g]:r2off[g + 1]] for g in range(NCH)]

            # fp8 (activation) stream first -- the act engine is the pacer
            dmas = [("8",) + r for r in g8] + [("16",) + r for r in g16]
            for kind, s, e in dmas:
                if kind == "8":
                    a, b = off8[s], off8[e]
                    if b > a:
                        nc.sync.dma_start(out=lt8[:, a:b],
                                          in_=packed8.ap()[:, a:b])
                else:
                    a, b = off16[s], off16[e]
                    if b > a:
                        nc.sync.dma_start(out=lt16[:, a:b],
                                          in_=packed16.ap()[:, a:b])

            psums = [pspool.tile([P, PIECE], f32, name=f"ps{b}", tag=f"ps{b}")
                     for b in range(NBANK)]
            for b in range(NBANK):
                nc.tensor.matmul(out=psums[b][0:NCH, :], lhsT=zcols,
                                 rhs=zmove, start=True, stop=False,
                                 skip_group_check=True)

            exp_fn = mybir.ActivationFunctionType.Exp
            for g in range(NCH):
                w, mg = W[g], m[g]
                if g in g16_first:                  # Schraudolph whole group
                    s_, e_ = g16_first[g]
                    a, b = off16[s_], off16[e_]
                    if b > a:
                        s16_i = s16[:].bitcast(i16)
                        nc.vector.tensor_scalar(
                            out=s16_i[:, a:b], in0=lt16[:, a:b],
                            scalar1=SCH_A, scalar2=SCH_B,
                            op0=mybir.AluOpType.mult, op1=mybir.AluOpType.add)
                if mg > 0:
                    a, b = off8[g], off8[g + 1]
                    nc.scalar.activation(out=s8[:, a:b], in_=lt8[:, a:b],
                                         func=exp_fn)
                    v = s8[:, a:b].rearrange("p (j two k) -> p j two k",
                                             two=2, k=w)
                    nc.vector.tensor_tensor(
                        out=rhs2[g][:, 0:mg * w].rearrange(
                            "p (j k) -> p j k", k=w),
                        in0=v[:, :, 0, :], in1=v[:, :, 1, :],
                        op=mybir.AluOpType.add)
                if mg < NPAIR:
                    a, b = off16[g], off16[g + 1]
                    v = s16[:, a:b].rearrange("p (j two k) -> p j two k",
                                              two=2, k=w)
                    nc.vector.tensor_tensor(
                        out=rhs2[g][:, mg * w:].rearrange(
                            "p (j k) -> p j k", k=w),
                        in0=v[:, :, 0, :], in1=v[:, :, 1, :],
                        op=mybir.AluOpType.add)

                ind = cst[:, g * NCH:(g + 1) * NCH]         # col g is ones
                for i in range(npieces[g]):
                    plen = min(PIECE, Lg[g] - i * PIECE)
                    nc.tensor.matmul(
                        out=psums[i][0:NCH, 0:plen],
                        lhsT=ind,
                        rhs=rhs2[g][:, i * PIECE:i * PIECE + plen],
                        start=False, stop=(last_chunk[i] == g),
                        skip_group_check=True)

                for b in range(NBANK):
                    if last_chunk[b] == g:
                        src = psums[b][0:NCH, :]
                        dst = stage[0:NCH, b * PIECE:(b + 1) * PIECE]
                        nc.scalar.copy(out=dst, in_=src)
                        nc.scalar.dma_start(
                            out=out.ap()[:, b * PIECE:(b + 1) * PIECE],
                            in_=dst)

    nc.compile()
    return nc


def _pack(logits, labels):
    perms, ls_all, W, m, off8, off16, L8, L16, g8, g16 = _plan(labels)
    x16 = logits.astype(np.float16)
    cst = np.zeros((P, NCH * NCH + PIECE), np.float16)
    for g in range(NCH):
        cst[:, g * NCH + g] = 1.0
    f8np = mybir.dt.np(fp8)
    in_maps, xs_list = [], []
    for i in range(NCORES):
        xs = x16[i * BC:(i + 1) * BC][perms[i]]
        pk8 = np.zeros((P, max(L8, 2)), f8np)
        pk16 = np.zeros((P, max(L16, 2)), np.float16)
        for g in range(NCH):
            w, mg = W[g], m[g]
            blk = xs[g * TPB * P:(g + 1) * TPB * P, :w]
            flat = blk.reshape(TPB, P, w).transpose(1, 0, 2).reshape(
                P, TPB * w)
            c0 = 2 * w * mg
            if c0 > 0:
                pk8[:, off8[g]:off8[g + 1]] = flat[:, :c0].astype(f8np)
            if c0 < TPB * w:
                pk16[:, off16[g]:off16[g + 1]] = flat[:, c0:]
        in_maps.append({"packed8": pk8, "packed16": pk16, "consts": cst})
        xs_list.append(xs)
    return in_maps, xs_list, ls_all, (W, m, off8, off16, L8, L16, g8, g16)


def _finish(outs, xs_list, ls_all, W, logits, labels, events):
    P2 = 2 * P                                     # pair-summed block rows
    NB = NT // 2
    sumexp = np.zeros(K, dtype=np.float64)
    for i in range(NCORES):
        cf = outs[i].astype(np.float64)            # (NCH, NBANK*PIECE)
        C = np.zeros((NB, K), dtype=np.float64)
        for g in range(NCH):
            w = W[g]
            C[g * NPAIR:(g + 1) * NPAIR, :w] = cf[g, :NPAIR * w].reshape(
                NPAIR, w)
        CC = np.cumsum(C, axis=0)
        ls = ls_all[i]
        hist = np.bincount(ls, minlength=K)
        count = np.cumsum(hist[::-1])[::-1]        # count[k] = #labels >= k
        xs = xs_list[i]
        for k in range(K):
            cnt = int(count[k])
            if cnt == 0:
                continue
            tb = cnt // P2
            if tb > 0:
                sumexp[k] += CC[tb - 1, k]
            if cnt % P2:
                sumexp[k] += np.exp(
                    xs[tb * P2:cnt, k].astype(np.float64)).sum()
    ev = events == 1
    own = logits[np.arange(B), labels].astype(np.float64)
    n_ev = np.bincount(labels[ev], minlength=K).astype(np.float64)
    numer = np.zeros(K)
    np.add.at(numer, labels[ev], own[ev])
    with np.errstate(divide="ignore"):
        denom_log = np.log(sumexp)
    terms = np.where(n_ev > 0, numer - n_ev * denom_log, 0.0)
    return np.array(-terms.sum() / max(ev.sum(), 1.0), dtype=np.float32)


def kernel(logits, labels, events, _trace=False):
    global LAST_EXEC_NS, LAST_TRACE, LAST_PROFILE_JSON
    logits = np.asarray(logits, dtype=np.float32)
    labels = np.asarray(labels, dtype=np.int32)
    events = np.asarray(events, dtype=np.int32)
    in_maps, xs_list, ls_all, plan = _pack(logits, labels)
    nc = build_nc(plan)
    try:
        res = run_bass_kernel_spmd(nc, in_maps, core_ids=list(range(NCORES)),
                                   trace=_trace)
    except Exception:
        # one retry: absorbs transient NRT device-unrecoverable hiccups
        res = run_bass_kernel_spmd(nc, in_maps, core_ids=list(range(NCORES)),
                                   trace=_trace)
    LAST_EXEC_NS = res.exec_time_ns
    LAST_TRACE = res.instructions_and_trace
    LAST_PROFILE_JSON = res.profile_json
    outs = [res.results[i]["out"] for i in range(NCORES)]
    return _finish(outs, xs_list, ls_all, plan[0], logits, labels, events)


# revision 20
# speedup vs baseline: 1.1313x; 1.1313x over previous
"""CoxTime loss kernel for 8 Trainium2 NeuronCores (v4: fp8 + pair-sum).

Host-side layout transform: each core's 32768-row logits shard is sorted
by label descending, so the risk set for column k is a row PREFIX of the
sorted shard.  Per 4096-row chunk g only columns 0..max_label(chunk) are
kept (truncation ~halves the work).  The widest chunks are assigned to
the Activation engine and shipped as fp8 e4m3 (native exp input, half
the HBM bytes); the rest ship as fp16 for the DVE's Schraudolph exp
(round(1477.32*x + 15301.06) as int16 IS fp16(~e^x), one 4x-mode
tensor_scalar).  A 2x-mode tensor_tensor then folds adjacent 128-row
tiles (pair-sum), halving the PE stream.  The PE reduces each chunk with
an indicator-column stationary into PSUM row g: C[256-row block, column]
partial sums.  Host finish: per column k, prefix-sum full blocks of C
plus an exact exp() correction for the <=255 boundary rows, then event
counts/numerators, the log and the scalar loss (O(B)+O(K) host work).
"""

import numpy as np

import concourse.bacc as bacc
import concourse.bass as bass
import concourse.mybir as mybir
import concourse.tile as tile
from concourse.bass_utils import run_bass_kernel_spmd

B = 262144
K = 128
NCORES = 8
BC = B // NCORES          # rows per core
P = 128                   # partitions
NT = BC // P              # 256 row-tiles per core
TPB = 16                  # row-tiles per chunk
NCH = NT // TPB           # 16 chunks per core
NPAIR = TPB // 2          # tile-pairs per chunk (pair-summed blocks)
NBANK = 2                 # PSUM banks: piece i of a chunk -> bank i
PIECE = 512               # moving-free elems per matmul (= 1 PSUM bank)

f32 = mybir.dt.float32
f16 = mybir.dt.float16
fp8 = mybir.dt.float8e4
i16 = mybir.dt.int16

# Schraudolph fp16 exp: i16 bits of fp16(2^t) ~= 1024*(t + 15 - 0.05757)
SCH_A = 1024.0 * 1.4426950408889634      # 1024*log2(e)
SCH_B = 1024.0 * (15.0 - 0.05757)

LAST_EXEC_NS = None
LAST_TRACE = None
LAST_PROFILE_JSON = None


def _plan(labels):
    """Per-core sort, shared chunk widths, act/DVE split, DMA groups."""
    perms, ls_all = [], []
    for i in range(NCORES):
        lab = labels[i * BC:(i + 1) * BC]
        perm = np.argsort(-lab, kind="stable")
        perms.append(perm)
        ls_all.append(lab[perm])
    ls_all = np.stack(ls_all)                      # (NCORES, BC) descending
    W = []
    for g in range(NCH):
        hi = int(ls_all[:, g * TPB * P:(g + 1) * TPB * P].max())
        W.append(min(K, hi + 1 + ((hi + 1) & 1)))  # round W up to even
    E = sum(TPB * w for w in W)                    # flat elems per partition
    # act (fp8) elem budget: balance measured act rate (1.044 ns/elem on
    # fp8) against the DVE's Schraudolph (0.316) + pair-sum (0.84) load
    A = (0.316 * E + 0.42 * E + 1200.0 - 2640.0) / 1.36
    m, rem = [], A
    for g in range(NCH):                           # widest chunks first
        mg = int(min(NPAIR, max(0.0, rem // (2 * W[g]))))
        m.append(mg)
        rem -= mg * 2 * W[g]
    off8 = np.cumsum([0] + [2 * W[g] * m[g] for g in range(NCH)]).tolist()
    off16 = np.cumsum([0] + [TPB * W[g] - 2 * W[g] * m[g]
                             for g in range(NCH)]).tolist()
    L8, L16 = off8[-1], off16[-1]

    def ranges(offs, nsplit):
        tot = offs[-1]
        cuts, res, s = [tot * (i + 1) / nsplit for i in range(nsplit)], [], 0
        for c in cuts:
            e = min(range(NCH + 1), key=lambda g: abs(offs[g] - c))
            if e > s:
                res.append((s, e))
                s = e
        if s < NCH:
            res.append((s, NCH))
        return res

    g8 = ranges(off8, 4)                           # fp8 DMA chunk ranges
    g16 = ranges(off16, 6)                         # fp16 DMA chunk ranges
    return perms, ls_all, W, m, off8, off16, L8, L16, g8, g16


def build_nc(plan):
    W, m, off8, off16, L8, L16, g8, g16 = plan
    nc = bacc.Bacc("TRN2", target_bir_lowering=False)
    packed8 = nc.declare_dram_parameter("packed8", [P, max(L8, 2)], fp8,
                                        isOutput=False)
    packed16 = nc.declare_dram_parameter("packed16", [P, max(L16, 2)], f16,
                                         isOutput=False)
    CW = NCH * NCH + PIECE
    consts = nc.declare_dram_parameter("consts", [P, CW], f16, isOutput=False)
    out = nc.declare_dram_parameter("out", [NCH, NBANK * PIECE], f16,
                                    isOutput=True)

    Lg = [NPAIR * W[g] for g in range(NCH)]        # pair-summed stream len
    npieces = [(Lg[g] + PIECE - 1) // PIECE for g in range(NCH)]
    last_chunk = [max(g for g in range(NCH) if npieces[g] > b)
                  for b in range(NBANK)]
    # emit each fp16 group's Schraudolph right before its first consumer
    # chunk (first chunk in range with a DVE part) so earlier act-chunk
    # pair-sums aren't head-of-line blocked on the DVE queue
    g16_first = {}
    for s, e in g16:
        first = next((g for g in range(s, e) if m[g] < NPAIR), s)
        g16_first[first] = (s, e)

    with tile.TileContext(nc) as tc:
        with (
            tc.tile_pool(name="const", bufs=1) as cpool,
            tc.tile_pool(name="psum", bufs=1, space="PSUM") as pspool,
        ):
            cst = cpool.tile([P, CW], f16)
            nc.sync.dma_start(out=cst[:], in_=consts.ap())
            zcols = cst[:, NCH * NCH:NCH * NCH + NCH]       # [P,16] zeros
            zmove = cst[:, NCH * NCH:NCH * NCH + PIECE]     # [P,512] zeros
            stage = cpool.tile([NCH, NBANK * PIECE], f16)

            lt8 = cpool.tile([P, max(L8, 2)], fp8)
            lt16 = cpool.tile([P, max(L16, 2)], f16)
            s8 = cpool.tile([P, max(L8, 2)], f16)           # act exp out
            s16 = cpool.tile([P, max(L16, 2)], f16)         # dve exp out
            rhs2all = cpool.tile([P, sum(Lg)], f16)
            r2off = np.cumsum([0] + Lg).tolist()
            rhs2 = [rhs2all[:, r2off[g]:r2off[g + 1]] for g in range(NCH)]

            # interleave: act never starves on fp8, fp16 streams alongside
            dmas = []
            for i in range(max(len(g8), len(g16))):
                if i < len(g8):
                    dmas.append(("8",) + g8[i])
                if i < len(g16):
                    dmas.append(("16",) + g16[i])
            for kind, s, e in dmas:
                if kind == "8":
                    a, b = off8[s], off8[e]
                    if b > a:
                        nc.sync.dma_start(out=lt8[:, a:b],
                                          in_=packed8.ap()[:, a:b])
                else:
                    a, b = off16[s], off16[e]
                    if b > a:
                        nc.sync.dma_start(out=lt16[:, a:b],
                                          in_=packed16.ap()[:, a:b])

            psums = [pspool.tile([P, PIECE], f32, name=f"ps{b}", tag=f"ps{b}")
                     for b in range(NBANK)]
            for b in range(NBANK):
                nc.tensor.matmul(out=psums[b][0:NCH, :], lhsT=zcols,
                                 rhs=zmove, start=True, stop=False,
                                 skip_group_check=True)

            exp_fn = mybir.ActivationFunctionType.Exp
            for g in range(NCH):
                w, mg = W[g], m[g]
                if g in g16_first:                  # Schraudolph whole group
                    s_, e_ = g16_first[g]
                    a, b = off16[s_], off16[e_]
                    if b > a:
                        s16_i = s16[:].bitcast(i16)
                        nc.vector.tensor_scalar(
                            out=s16_i[:, a:b], in0=lt16[:, a:b],
                            scalar1=SCH_A, scalar2=SCH_B,
                            op0=mybir.AluOpType.mult, op1=mybir.AluOpType.add)
                if mg > 0:
                    a, b = off8[g], off8[g + 1]
                    nc.scalar.activation(out=s8[:, a:b], in_=lt8[:, a:b],
                                         func=exp_fn)
                    v = s8[:, a:b].rearrange("p (j two k) -> p j two k",
                                             two=2, k=w)
                    nc.vector.tensor_tensor(
                        out=rhs2[g][:, 0:mg * w].rearrange(
                            "p (j k) -> p j k", k=w),
                        in0=v[:, :, 0, :], in1=v[:, :, 1, :],
                        op=mybir.AluOpType.add)
                if mg < NPAIR:
                    a, b = off16[g], off16[g + 1]
                    v = s16[:, a:b].rearrange("p (j two k) -> p j two k",
                                              two=2, k=w)
                    nc.vector.tensor_tensor(
                        out=rhs2[g][:, mg * w:].rearrange(
                            "p (j k) -> p j k", k=w),
                        in0=v[:, :, 0, :], in1=v[:, :, 1, :],
                        op=mybir.AluOpType.add)

                ind = cst[:, g * NCH:(g + 1) * NCH]         # col g is ones
                for i in range(npieces[g]):
                    plen = min(PIECE, Lg[g] - i * PIECE)
                    nc.tensor.matmul(
                        out=psums[i][0:NCH, 0:plen],
                        lhsT=ind,
                        rhs=rhs2[g][:, i * PIECE:i * PIECE + plen],
                        start=False, stop=(last_chunk[i] == g),
                        skip_group_check=True)

                for b in range(NBANK):
                    if last_chunk[b] == g:
                        src = psums[b][0:NCH, :]
                        dst = stage[0:NCH, b * PIECE:(b + 1) * PIECE]
                        nc.scalar.copy(out=dst, in_=src)
                        nc.scalar.dma_start(
                            out=out.ap()[:, b * PIECE:(b + 1) * PIECE],
                            in_=dst)

    nc.compile()
    return nc


def _pack(logits, labels):
    perms, ls_all, W, m, off8, off16, L8, L16, g8, g16 = _plan(labels)
    x16 = logits.astype(np.float16)
    cst = np.zeros((P, NCH * NCH + PIECE), np.float16)
    for g in range(NCH):
        cst[:, g * NCH + g] = 1.0
    f8np = mybir.dt.np(fp8)
    in_maps, xs_list = [], []
    for i in range(NCORES):
        xs = x16[i * BC:(i + 1) * BC][perms[i]]
        pk8 = np.zeros((P, max(L8, 2)), f8np)
        pk16 = np.zeros((P, max(L16, 2)), np.float16)
        for g in range(NCH):
            w, mg = W[g], m[g]
            blk = xs[g * TPB * P:(g + 1) * TPB * P, :w]
            flat = blk.reshape(TPB, P, w).transpose(1, 0, 2).reshape(
                P, TPB * w)
            c0 = 2 * w * mg
            if c0 > 0:
                pk8[:, off8[g]:off8[g + 1]] = flat[:, :c0].astype(f8np)
            if c0 < TPB * w:
                pk16[:, off16[g]:off16[g + 1]] = flat[:, c0:]
        in_maps.append({"packed8": pk8, "packed16": pk16, "consts": cst})
        xs_list.append(xs)
    return in_maps, xs_list, ls_all, (W, m, off8, off16, L8, L16, g8, g16)


def _finish(outs, xs_list, ls_all, W, logits, labels, events):
    P2 = 2 * P                                     # pair-summed block rows
    NB = NT // 2
    sumexp = np.zeros(K, dtype=np.float64)
    for i in range(NCORES):
        cf = outs[i].astype(np.float64)            # (NCH, NBANK*PIECE)
        C = np.zeros((NB, K), dtype=np.float64)
        for g in range(NCH):
            w = W[g]
            C[g * NPAIR:(g + 1) * NPAIR, :w] = cf[g, :NPAIR * w].reshape(
                NPAIR, w)
        CC = np.cumsum(C, axis=0)
        ls = ls_all[i]
        hist = np.bincount(ls, minlength=K)
        count = np.cumsum(hist[::-1])[::-1]        # count[k] = #labels >= k
        xs = xs_list[i]
        for k in range(K):
            cnt = int(count[k])
            if cnt == 0:
                continue
            tb = cnt // P2
            if tb > 0:
                sumexp[k] += CC[tb - 1, k]
            if cnt % P2:
                sumexp[k] += np.exp(
                    xs[tb * P2:cnt, k].astype(np.float64)).sum()
    ev = events == 1
    own = logits[np.arange(B), labels].astype(np.float64)
    n_ev = np.bincount(labels[ev], minlength=K).astype(np.float64)
    numer = np.zeros(K)
    np.add.at(numer, labels[ev], own[ev])
    with np.errstate(divide="ignore"):
        denom_log = np.log(sumexp)
    terms = np.where(n_ev > 0, numer - n_ev * denom_log, 0.0)
    return np.array(-terms.sum() / max(ev.sum(), 1.0), dtype=np.float32)


def kernel(logits, labels, events, _trace=False):
    global LAST_EXEC_NS, LAST_TRACE, LAST_PROFILE_JSON
    logits = np.asarray(logits, dtype=np.float32)
    labels = np.asarray(labels, dtype=np.int32)
    events = np.asarray(events, dtype=np.int32)
    in_maps, xs_list, ls_all, plan = _pack(logits, labels)
    nc = build_nc(plan)
    try:
        res = run_bass_kernel_spmd(nc, in_maps, core_ids=list(range(NCORES)),
                                   trace=_trace)
    except Exception:
        # one retry: absorbs transient NRT device-unrecoverable hiccups
        res = run_bass_kernel_spmd(nc, in_maps, core_ids=list(range(NCORES)),
                                   trace=_trace)
    LAST_EXEC_NS = res.exec_time_ns
    LAST_TRACE = res.instructions_and_trace
    LAST_PROFILE_JSON = res.profile_json
    outs = [res.results[i]["out"] for i in range(NCORES)]
    return _finish(outs, xs_list, ls_all, plan[0], logits, labels, events)
